# revision 21
# baseline (speedup 1.0000x reference)
"""Trainium2 Bass kernel for a pre-norm transformer decoder layer.

Full inputs in, full output out. 8-way data-parallel over tokens
(batch 2 x 4 query-slices of 512). Each core redundantly computes K/V
for its batch's full 2048-token sequence and owns 512 query tokens.

v2 design:
- Host precomputes LN1 (mu/rstd) and uploads z8 = fp8(norm(x)) in a
  feature-major layout, plus fp8 DoubleRow-interleaved attention
  weights (2x tensor throughput on contraction >= 256).
- Attention entirely in fp8: Q/K/V/out-proj via fp8 DoubleRow matmuls;
  exp(scores) written as fp8 with a -2 bias (cancels in softmax);
  PV contracts 256 keys per DoubleRow matmul with an appended
  ones-column accumulating the softmax denominator.
- bk cancels in softmax (per-query shift); bv folds into bo; bq rides
  the PSUM-drain bias; ln biases fold into bq/b1 (host).
- MLP stays bf16 (fp8 there costs ~1.7e-2 rel err; attention fp8 costs
  ~1e-3). LN2 stats via ones-matmul over feature-major x_res.
- Two-stage token pipeline (256+256) overlaps wo/LN2/MLP tensor work
  with the scalar-engine-bound exp stream of the next attention half.

Shapes: x (2, 2048, 1024), 16 heads, dk=64, d_ff=2048, eps=1e-5.
"""
import os
import threading

import numpy as np
import ml_dtypes

import concourse.mybir as mybir
import concourse.tile as tile
from concourse import bacc
from concourse.bass_utils import run_bass_kernel_spmd
from contextlib import ExitStack

F32 = mybir.dt.float32
BF16 = mybir.dt.bfloat16
FP8 = mybir.dt.float8e4
AF = mybir.ActivationFunctionType
OP = mybir.AluOpType
DR = mybir.MatmulPerfMode.DoubleRow

B, S, D = 2, 2048, 1024
H, DK, FF = 16, 64, 2048
EPS = 1e-5
NCORES = 8
SQ = S * B // NCORES          # 512 own query tokens per core
ND = D // 128                 # 8 feature chunks
NT = S // 128                 # 16 key-token tiles
NF = FF // 128                # 16 ff chunks
NHT = 4                       # o8 tiles (4 heads each)

_BF = ml_dtypes.bfloat16
_F8 = ml_dtypes.float8_e4m3fn


def _build_nc():
    nc = bacc.Bacc("TRN2", target_bir_lowering=False, debug=False,
                   num_devices=NCORES)

    z8d = [nc.dram_tensor(f"z8q{c}", [128, ND, 512], FP8,
                          kind="ExternalInput").ap() for c in range(4)]
    xfmd = nc.dram_tensor("xfm", [128, ND, SQ], BF16,
                          kind="ExternalInput").ap()
    wq8d = nc.dram_tensor("wq8", [128, 4, 2, D], FP8,
                          kind="ExternalInput").ap()
    wk8d = nc.dram_tensor("wk8", [128, 4, 2, D], FP8,
                          kind="ExternalInput").ap()
    wv8d = nc.dram_tensor("wv8", [128, 4, 2, D], FP8,
                          kind="ExternalInput").ap()
    wo8d = nc.dram_tensor("wo8", [128, 4, 2, D], FP8,
                          kind="ExternalInput").ap()
    w1bd = nc.dram_tensor("w1b", [128, ND, FF], BF16,
                          kind="ExternalInput").ap()
    w2bd = nc.dram_tensor("w2b", [128, NF, D], BF16,
                          kind="ExternalInput").ap()
    bqd = nc.dram_tensor("bq", [128, ND], F32, kind="ExternalInput").ap()
    bod = nc.dram_tensor("bo", [128, ND], F32, kind="ExternalInput").ap()
    b1d = nc.dram_tensor("b1", [128, NF], F32, kind="ExternalInput").ap()
    b2d = nc.dram_tensor("b2", [128, ND], F32, kind="ExternalInput").ap()
    outd = nc.dram_tensor("out", [SQ, D], F32, kind="ExternalOutput").ap()
    dbg = os.environ.get("KDBG", "0") == "1"
    if dbg:
        dqd = nc.dram_tensor("dq", [128, ND, SQ], BF16,
                             kind="ExternalOutput").ap()
        dkd = nc.dram_tensor("dk", [128, ND, S], BF16,
                             kind="ExternalOutput").ap()
        dvd = nc.dram_tensor("dv", [128, 2, H, DK + 1], FP8,
                             kind="ExternalOutput").ap()
        dod = nc.dram_tensor("do", [NHT, 128, 2, SQ], FP8,
                             kind="ExternalOutput").ap()
        dxd = nc.dram_tensor("dx", [128, ND, 256], BF16,
                             kind="ExternalOutput").ap()
        dzd = nc.dram_tensor("dz", [128, ND, 256], BF16,
                             kind="ExternalOutput").ap()
        dhd = nc.dram_tensor("dh", [128, NF, 256], BF16,
                             kind="ExternalOutput").ap()

    with tile.TileContext(nc) as tc, ExitStack() as ctx:
        const = ctx.enter_context(tc.tile_pool(name="const", bufs=1))

        eps_sb = const.tile([1, 1], F32, tag="eps")
        nc.vector.memset(eps_sb, EPS)
        nbias = const.tile([128, 1], F32, tag="nbias")
        nc.vector.memset(nbias, -2.0)
        ones_bf = const.tile([128, 1], BF16, tag="ones")
        nc.vector.memset(ones_bf, 1.0)

        ctxMLP = ExitStack()
        op8 = ctxMLP.enter_context(tc.tile_pool(name="op8", bufs=1))
        o8 = [op8.tile([128, 2, SQ], FP8, tag=f"o{t}", name=f"o8_{t}")
              for t in range(NHT)]

        # q/k feature-major bf16; v bf16 token-major (+ones col)
        ctxQK = ExitStack()
        qkp = ctxQK.enter_context(tc.tile_pool(name="qkp", bufs=1))
        vp = ctxQK.enter_context(tc.tile_pool(name="vp", bufs=1))
        q_dr = [qkp.tile([128, 2, SQ], FP8, tag=f"qz{h}", name=f"qz{h}")
                for h in range(H)]
        k_fm = qkp.tile([128, ND, S], FP8, tag="k", name="k_fm")
        v_all = vp.tile([128, NT, H, DK + 1], BF16, tag="v",
                         name="v_all")

        ctxZW = ExitStack()
        zp = ctxZW.enter_context(tc.tile_pool(name="zp", bufs=1))
        wA = ctxZW.enter_context(tc.tile_pool(name="wA", bufs=1))
        # staged input loads: own z first (Q), then the rest
        z8 = [zp.tile([128, ND, 512], FP8, tag=f"z8q{c}", name=f"z8q{c}")
              for c in range(4)]
        nc.sync.dma_start(out=z8[0][:, 0:4, :], in_=z8d[0][:, 0:4, :])
        nc.scalar.dma_start(out=z8[0][:, 4:8, :], in_=z8d[0][:, 4:8, :])
        nc.sync.dma_start(out=z8[1], in_=z8d[1])
        nc.scalar.dma_start(out=z8[2], in_=z8d[2])
        nc.sync.dma_start(out=z8[3], in_=z8d[3])
        bq_sb = const.tile([128, ND], F32, tag="bq")
        nc.scalar.dma_start(out=bq_sb, in_=bqd)
        bo_sb = const.tile([128, ND], F32, tag="bo")
        nc.scalar.dma_start(out=bo_sb, in_=bod)
        b1_sb = const.tile([128, NF], F32, tag="b1")
        nc.scalar.dma_start(out=b1_sb, in_=b1d)
        b2_sb = const.tile([128, ND], F32, tag="b2")
        nc.scalar.dma_start(out=b2_sb, in_=b2d)

        wq8 = wA.tile([128, 4, 2, D], FP8, tag="wq8", name="wq8")
        nc.gpsimd.dma_start(out=wq8, in_=wq8d)
        wk8 = wA.tile([128, 4, 2, D], FP8, tag="wk8", name="wk8")
        nc.gpsimd.dma_start(out=wk8, in_=wk8d)
        wv8 = wA.tile([128, 4, 2, D], FP8, tag="wv8", name="wv8")
        nc.gpsimd.dma_start(out=wv8, in_=wv8d)
        wo8 = const.tile([128, 4, 2, D], FP8, tag="wo8", name="wo8")
        nc.gpsimd.dma_start(out=wo8, in_=wo8d)
        xfm = const.tile([128, ND, SQ], BF16, tag="xfm", name="xfm")
        nc.gpsimd.dma_start(out=xfm, in_=xfmd)

        ctxQKV = ExitStack()
        psA = ctxQKV.enter_context(tc.tile_pool(name="psA", bufs=4,
                                                space="PSUM"))

        # Q: own 512 tokens, fp8 DoubleRow; drains write per-head
        # zero-padded tiles so scores contract K=128 (no PE tile-config
        # switches vs the K=128 PV matmuls)
        for h in range(H):
            nc.gpsimd.memset(q_dr[h], 0.0)
        for j in range(ND):
            pq = psA.tile([128, 2, 512], F32, tag="ps")
            for c in range(4):
                nc.tensor.matmul(pq[:, 0, :],
                                 wq8[:, c, :, j * 128:(j + 1) * 128],
                                 z8[0][:, 2 * c:2 * c + 2, :],
                                 start=(c == 0), stop=(c == 3), perf_mode=DR)
            for s_ in range(2):
                h = 2 * j + s_
                nc.vector.tensor_scalar(
                    q_dr[h][64 * s_:64 * s_ + 64, j % 2, :],
                    pq[64 * s_:64 * s_ + 64, 0, :],
                    bq_sb[64 * s_:64 * s_ + 64, j:j + 1], None, op0=OP.add)

        # K: all 2048 tokens by quadrant; paired drains (no bias: bk
        # cancels per-query in softmax)
        for cq in range(4):
            for a in range(ND // 2):
                pk = psA.tile([128, 2, 512], F32, tag="ps")
                for half in range(2):
                    j = 2 * a + half
                    for c in range(4):
                        nc.tensor.matmul(
                            pk[:, half, :],
                            wk8[:, c, :, j * 128:(j + 1) * 128],
                            z8[cq][:, 2 * c:2 * c + 2, :],
                            start=(c == 0), stop=(c == 3), perf_mode=DR)
                nc.vector.tensor_copy(
                    k_fm[:, 2 * a:2 * a + 2, cq * 512:(cq + 1) * 512], pk)

        # V: token-major [tok, h, dk] bf16; paired drains
        nc.gpsimd.memset(v_all[:, :, :, DK:DK + 1], 1.0)
        for c in range(NT // 2):
            for half in range(2):
                pv = psA.tile([128, 2, 512], F32, tag="ps")
                for b_ in range(2):
                    t = 2 * c + b_
                    for d in range(4):
                        nc.tensor.matmul(
                            pv[:, b_, :],
                            z8[t // 4][:, 2 * d:2 * d + 2,
                                       (t % 4) * 128:(t % 4 + 1) * 128],
                            wv8[:, d, :, half * 512:(half + 1) * 512],
                            start=(d == 0), stop=(d == 3), perf_mode=DR)
                nc.vector.tensor_copy(
                    v_all[:, 2 * c:2 * c + 2, 8 * half:8 * half + 8, 0:DK],
                    pv.rearrange("p b (h d) -> p b h d", h=8))
        ctxQKV.close()
        ctxZW.close()

        # ---- attention: 16 heads, fp8 exp + DoubleRow PV ----
        ctxAPS = ExitStack()
        pgp = ctxAPS.enter_context(tc.tile_pool(name="pgp", bufs=3,
                                                space="PSUM"))
        ppvp = ctxAPS.enter_context(tc.tile_pool(name="ppvp", bufs=2,
                                                 space="PSUM"))
        stp = ctxAPS.enter_context(tc.tile_pool(name="stp", bufs=4))

        w1b = const.tile([128, ND, FF], BF16, tag="w1b", name="w1b")
        nc.gpsimd.dma_start(out=w1b, in_=w1bd)
        w2b = const.tile([128, NF, D], BF16, tag="w2b", name="w2b")
        nc.gpsimd.dma_start(out=w2b, in_=w2bd)

        for h in range(H):
            a2 = 2 * (h // 4)
            ppv = ppvp.tile([DK + 1, SQ], F32, tag="ppv", name=f"ppv{h}")
            for c in range(NT // 2):
                pg = pgp.tile([128, 2, 512], F32, tag="pg")
                for b_ in range(2):
                    kt = 2 * c + b_
                    nc.tensor.matmul(
                        pg[:, b_, :],
                        k_fm[:, a2:a2 + 2, kt * 128:(kt + 1) * 128],
                        q_dr[h], start=True, stop=True, perf_mode=DR)
                st8 = stp.tile([128, 2, 512], BF16, tag="st")
                nc.scalar.activation(st8, pg, AF.Exp, bias=nbias, scale=0.125)
                for b_ in range(2):
                    nc.tensor.matmul(ppv, v_all[:, 2 * c + b_, h, :],
                                     st8[:, b_, :],
                                     start=(c == 0 and b_ == 0),
                                     stop=(c == NT // 2 - 1 and b_ == 1))
            den_c = stp.tile([1, SQ], F32, tag="denc", bufs=2)
            nc.vector.tensor_copy(den_c, ppv[DK:DK + 1, :])
            den_r = stp.tile([1, SQ], F32, tag="denr", bufs=2)
            nc.vector.reciprocal_approx_fast(den_r, den_c)
            rb = stp.tile([DK, SQ], F32, tag="rb", bufs=2)
            nc.gpsimd.partition_broadcast(rb, den_r)
            nc.vector.tensor_mul(
                o8[h // 4][64 * (h % 2):64 * (h % 2) + 64, (h // 2) % 2, :],
                ppv[0:DK, :], rb)

        ctxAPS.close()
        ctxQK.close()

        # ---- two token-halves: wo -> LN2 -> MLP pipeline ----
        psB = ctxMLP.enter_context(tc.tile_pool(name="psB", bufs=4,
                                                space="PSUM"))
        psST = ctxMLP.enter_context(tc.tile_pool(name="psST", bufs=2,
                                                 space="PSUM"))
        xrp = ctxMLP.enter_context(tc.tile_pool(name="xrp", bufs=1))
        lns = ctxMLP.enter_context(tc.tile_pool(name="lns", bufs=2))
        outp = ctxMLP.enter_context(tc.tile_pool(name="outp", bufs=1))
        x_res = [xrp.tile([128, ND, 256], BF16, tag=f"xr{s_}",
                          name=f"xres{s_}") for s_ in range(2)]
        xsq = [xrp.tile([128, ND, 256], BF16, tag=f"xq{s_}",
                        name=f"xsq{s_}") for s_ in range(2)]
        z2 = [xrp.tile([128, ND, 256], BF16, tag=f"z2{s_}",
                       name=f"z2_{s_}") for s_ in range(2)]
        h_sb = [xrp.tile([128, NF, 256], BF16, tag=f"h{s_}",
                         name=f"h_{s_}") for s_ in range(2)]
        out_tm = [outp.tile([128, 2, D], BF16, tag=f"otm{s_}",
                            name=f"out_tm{s_}") for s_ in range(2)]

        def wo_half(s_):
            lo = 256 * s_
            for o in range(ND):
                py = psB.tile([128, 256], F32, tag="psb")
                for c in range(NHT):
                    nc.tensor.matmul(py, wo8[:, c, :, o * 128:(o + 1) * 128],
                                     o8[c][:, :, lo:lo + 256],
                                     start=(c == 0), stop=(c == NHT - 1),
                                     perf_mode=DR)
                nc.vector.scalar_tensor_tensor(
                    x_res[s_][:, o, :], py, bo_sb[:, o:o + 1],
                    xfm[:, o, lo:lo + 256], op0=OP.add, op1=OP.add)
                nc.vector.tensor_mul(xsq[s_][:, o, :], x_res[s_][:, o, :],
                                     x_res[s_][:, o, :])

        stat_ps = {}

        def stats_half(s_):
            psum1 = psST.tile([1, 256], F32, tag="s1", bufs=2)
            pseq = psST.tile([1, 256], F32, tag="s2", bufs=2)
            stat_ps[s_] = (psum1, pseq)
            for o in range(ND):
                nc.tensor.matmul(psum1, ones_bf, x_res[s_][:, o, :],
                                 start=(o == 0), stop=(o == ND - 1))
            for o in range(ND):
                nc.tensor.matmul(pseq, ones_bf, xsq[s_][:, o, :],
                                 start=(o == 0), stop=(o == ND - 1))

        def ln2_head_half(s_):
            psum1, pseq = stat_ps[s_]
            mu = lns.tile([1, 256], F32, tag="mu", bufs=1)
            nc.vector.tensor_scalar(mu, psum1, 1.0 / D, None, op0=OP.mult)
            musq = lns.tile([1, 256], F32, tag="musq", bufs=1)
            nc.vector.tensor_mul(musq, mu, mu)
            var = lns.tile([1, 256], F32, tag="var", bufs=1)
            nc.vector.scalar_tensor_tensor(var, pseq, 1.0 / D, musq,
                                           op0=OP.mult, op1=OP.subtract)
            sq = lns.tile([1, 256], F32, tag="sq", bufs=1)
            nc.scalar.activation(sq, var, AF.Sqrt, bias=eps_sb, scale=1.0)
            rstd = lns.tile([1, 256], F32, tag="rstd", bufs=1)
            nc.vector.reciprocal(rstd, sq)
            mu_b = lns.tile([1, 256], BF16, tag="mub")
            nc.vector.tensor_copy(mu_b, mu)
            rstd_b = lns.tile([1, 256], BF16, tag="rstdb")
            nc.vector.tensor_copy(rstd_b, rstd)
            mu_bc = lns.tile([128, 256], BF16, tag="mubc")
            nc.gpsimd.partition_broadcast(mu_bc, mu_b)
            rstd_bc = lns.tile([128, 256], BF16, tag="rstdbc")
            nc.gpsimd.partition_broadcast(rstd_bc, rstd_b)
            nc.vector.tensor_sub(
                z2[s_], x_res[s_],
                mu_bc.rearrange("p (o t) -> p o t", o=1).broadcast_to(
                    [128, ND, 256]))
            nc.vector.tensor_mul(
                z2[s_], z2[s_],
                rstd_bc.rearrange("p (o t) -> p o t", o=1).broadcast_to(
                    [128, ND, 256]))

        def mlp_body_half(s_):
            lo = 256 * s_
            # MLP1 + relu(+b1) -> bf16 h
            for f in range(NF):
                ph = psB.tile([128, 256], F32, tag="psb")
                for d in range(ND):
                    nc.tensor.matmul(ph, w1b[:, d, f * 128:(f + 1) * 128],
                                     z2[s_][:, d, :],
                                     start=(d == 0), stop=(d == ND - 1))
                nc.scalar.activation(h_sb[s_][:, f, :], ph, AF.Relu,
                                     bias=b1_sb[:, f:f + 1], scale=1.0)
            # MLP2 + b2 + x_res -> bf16 out_fm -> transpose to token-major
            for o in range(ND):
                p2 = psB.tile([128, 256], F32, tag="psb")
                for f in range(NF):
                    nc.tensor.matmul(p2, w2b[:, f, o * 128:(o + 1) * 128],
                                     h_sb[s_][:, f, :],
                                     start=(f == 0), stop=(f == NF - 1))
                ofm = lns.tile([128, 256], BF16, tag="ofm", bufs=2)
                nc.vector.scalar_tensor_tensor(
                    ofm, p2, b2_sb[:, o:o + 1], x_res[s_][:, o, :],
                    op0=OP.add, op1=OP.add)
                nc.sync.dma_start_transpose(
                    out_tm[s_][:, :, o * 128:(o + 1) * 128], ofm)
            for tt in range(2):
                t = 2 * s_ + tt
                o_st = outp.tile([128, D], F32, tag="ost", bufs=2)
                nc.vector.tensor_copy(o_st, out_tm[s_][:, tt, :])
                eng = nc.sync if t % 2 == 0 else nc.gpsimd
                eng.dma_start(out=outd[t * 128:(t + 1) * 128, :], in_=o_st)

        wo_half(0)
        stats_half(0)
        ln2_head_half(0)
        wo_half(1)
        stats_half(1)
        mlp_body_half(0)
        ln2_head_half(1)
        mlp_body_half(1)

        _dbg_marker = True
        if dbg:
            nc.sync.dma_start(out=dqd[:, 0:2, :].rearrange("p a t -> p (a t)"), in_=q_z[0])
            nc.sync.dma_start(out=dkd, in_=k_fm)
            nc.sync.dma_start(out=dvd, in_=v_all[:, 0:2, :, :])
            for t in range(NHT):
                nc.sync.dma_start(out=dod[t], in_=o8[t])
            nc.sync.dma_start(out=dxd, in_=x_res[0])
            nc.sync.dma_start(out=dzd, in_=z2[0])
            nc.sync.dma_start(out=dhd, in_=h_sb[0])
        ctxMLP.close()

    nc.compile()
    return nc


_LOCK = threading.Lock()
_NC = None


def _get_nc():
    global _NC
    with _LOCK:
        if _NC is None:
            _NC = _build_nc()
    return _NC


def _prep_inputs(inputs):
    x = np.asarray(inputs["x"], np.float32)
    g1 = np.asarray(inputs["ln1_g"], np.float32)
    lb1 = np.asarray(inputs["ln1_b"], np.float32)
    g2 = np.asarray(inputs["ln2_g"], np.float32)
    lb2 = np.asarray(inputs["ln2_b"], np.float32)
    wq = np.asarray(inputs["wq"], np.float32)
    wk = np.asarray(inputs["wk"], np.float32)
    wv = np.asarray(inputs["wv"], np.float32)
    wo = np.asarray(inputs["wo"], np.float32)
    w1 = np.asarray(inputs["w1"], np.float32)
    w2 = np.asarray(inputs["w2"], np.float32)

    def dr8(wt):
        # [D_in, D_out] -> [128, 4, 2, D_out] fp8 DoubleRow layout
        return np.ascontiguousarray(
            wt.reshape(4, 2, 128, D).transpose(2, 0, 1, 3)).astype(_F8)

    # host LN1 + fp8 quantize, feature-major
    mu = x.mean(-1, keepdims=True)
    var = x.var(-1, keepdims=True)
    z = (x - mu) / np.sqrt(var + EPS)          # [B, S, D]
    z8 = z.transpose(0, 2, 1).astype(_F8)      # [B, D, S] feature-major
    xfm_all = x.transpose(0, 2, 1).astype(_BF)  # [B, D, S]

    shared = {
        "wq8": dr8(g1[:, None] * wq.T),
        "wk8": dr8(g1[:, None] * wk.T),
        "wv8": dr8(g1[:, None] * wv.T),
        "wo8": dr8(wo.T),
        "w1b": np.ascontiguousarray(
            (g2[:, None] * w1.T).reshape(ND, 128, FF).transpose(
                1, 0, 2)).astype(_BF),
        "w2b": np.ascontiguousarray(
            w2.T.reshape(NF, 128, D).transpose(1, 0, 2)).astype(_BF),
        "bq": np.ascontiguousarray(
            (np.asarray(inputs["bq"], np.float32) + wq @ lb1).reshape(
                ND, 128).T),
        "bo": np.ascontiguousarray(
            (np.asarray(inputs["bo"], np.float32)
             + wo @ np.asarray(inputs["bv"], np.float32)).reshape(
                 ND, 128).T),
        "b1": np.ascontiguousarray(
            (np.asarray(inputs["b1"], np.float32) + w1 @ lb2).reshape(
                NF, 128).T),
        "b2": np.ascontiguousarray(
            np.asarray(inputs["b2"], np.float32).reshape(ND, 128).T),
    }

    in_maps = []
    for core in range(NCORES):
        b = core // (NCORES // B)
        qoff = (core % (NCORES // B)) * SQ
        zb = z8[b]                              # [D, S] fp8
        # own 512 tokens first, then the rest (key order is softmax-inv)
        perm = np.concatenate(
            [np.arange(qoff, qoff + SQ), np.arange(0, qoff),
             np.arange(qoff + SQ, S)])
        zperm = zb[:, perm]                     # [D, S]
        m = dict(shared)
        for c in range(4):
            m[f"z8q{c}"] = np.ascontiguousarray(
                zperm[:, c * 512:(c + 1) * 512].reshape(ND, 128, 512)
                .transpose(1, 0, 2))
        m["xfm"] = np.ascontiguousarray(
            xfm_all[b][:, qoff:qoff + SQ].reshape(ND, 128, SQ)
            .transpose(1, 0, 2))
        in_maps.append(m)
    return in_maps


def _run(inputs, trace=False, tmpdir=None):
    nc = _get_nc()
    in_maps = _prep_inputs(inputs)
    res = run_bass_kernel_spmd(nc, in_maps, core_ids=list(range(NCORES)),
                               trace=trace, tmpdir=tmpdir)
    out = np.empty((B, S, D), np.float32)
    for core in range(NCORES):
        b = core // (NCORES // B)
        qoff = (core % (NCORES // B)) * SQ
        out[b, qoff:qoff + SQ] = res.results[core]["out"]
    return out, res


def kernel(**inputs):
    out, _ = _run(inputs, trace=False)
    return out


# revision 22
# speedup vs baseline: 1.0005x; 1.0005x over previous
"""Trainium2 Bass kernel for a pre-norm transformer decoder layer.

Full inputs in, full output out. 8-way data-parallel over tokens
(batch 2 x 4 query-slices of 512). Each core redundantly computes K/V
for its batch's full 2048-token sequence and owns 512 query tokens.

v2 design:
- Host precomputes LN1 (mu/rstd) and uploads z8 = fp8(norm(x)) in a
  feature-major layout, plus fp8 DoubleRow-interleaved attention
  weights (2x tensor throughput on contraction >= 256).
- Attention entirely in fp8: Q/K/V/out-proj via fp8 DoubleRow matmuls;
  exp(scores) written as fp8 with a -2 bias (cancels in softmax);
  PV contracts 256 keys per DoubleRow matmul with an appended
  ones-column accumulating the softmax denominator.
- bk cancels in softmax (per-query shift); bv folds into bo; bq rides
  the PSUM-drain bias; ln biases fold into bq/b1 (host).
- MLP stays bf16 (fp8 there costs ~1.7e-2 rel err; attention fp8 costs
  ~1e-3). LN2 stats via ones-matmul over feature-major x_res.
- Two-stage token pipeline (256+256) overlaps wo/LN2/MLP tensor work
  with the scalar-engine-bound exp stream of the next attention half.

Shapes: x (2, 2048, 1024), 16 heads, dk=64, d_ff=2048, eps=1e-5.
"""
import os
import threading

import numpy as np
import ml_dtypes

import concourse.mybir as mybir
import concourse.tile as tile
from concourse import bacc
from concourse.bass_utils import run_bass_kernel_spmd
from contextlib import ExitStack

F32 = mybir.dt.float32
BF16 = mybir.dt.bfloat16
FP8 = mybir.dt.float8e4
AF = mybir.ActivationFunctionType
OP = mybir.AluOpType
DR = mybir.MatmulPerfMode.DoubleRow

B, S, D = 2, 2048, 1024
H, DK, FF = 16, 64, 2048
EPS = 1e-5
NCORES = 8
SQ = S * B // NCORES          # 512 own query tokens per core
ND = D // 128                 # 8 feature chunks
NT = S // 128                 # 16 key-token tiles
NF = FF // 128                # 16 ff chunks
NHT = 4                       # o8 tiles (4 heads each)

_BF = ml_dtypes.bfloat16
_F8 = ml_dtypes.float8_e4m3fn


def _build_nc():
    nc = bacc.Bacc("TRN2", target_bir_lowering=False, debug=False,
                   num_devices=NCORES)

    z8d = [nc.dram_tensor(f"z8q{c}", [128, ND, 512], FP8,
                          kind="ExternalInput").ap() for c in range(4)]
    xfmd = nc.dram_tensor("xfm", [128, ND, SQ], BF16,
                          kind="ExternalInput").ap()
    wq8d = nc.dram_tensor("wq8", [128, 4, 2, D], FP8,
                          kind="ExternalInput").ap()
    wk8d = nc.dram_tensor("wk8", [128, 4, 2, D], FP8,
                          kind="ExternalInput").ap()
    wv8d = nc.dram_tensor("wv8", [128, 4, 2, D], FP8,
                          kind="ExternalInput").ap()
    wo8d = nc.dram_tensor("wo8", [128, 4, 2, D], FP8,
                          kind="ExternalInput").ap()
    w1bd = nc.dram_tensor("w1b", [128, ND, FF], BF16,
                          kind="ExternalInput").ap()
    w2bd = nc.dram_tensor("w2b", [128, NF, D], BF16,
                          kind="ExternalInput").ap()
    bqd = nc.dram_tensor("bq", [128, ND], F32, kind="ExternalInput").ap()
    bod = nc.dram_tensor("bo", [128, ND], F32, kind="ExternalInput").ap()
    b1d = nc.dram_tensor("b1", [128, NF], F32, kind="ExternalInput").ap()
    b2d = nc.dram_tensor("b2", [128, ND], F32, kind="ExternalInput").ap()
    outd = nc.dram_tensor("out", [SQ, D], F32, kind="ExternalOutput").ap()
    dbg = os.environ.get("KDBG", "0") == "1"
    if dbg:
        dqd = nc.dram_tensor("dq", [128, ND, SQ], BF16,
                             kind="ExternalOutput").ap()
        dkd = nc.dram_tensor("dk", [128, ND, S], BF16,
                             kind="ExternalOutput").ap()
        dvd = nc.dram_tensor("dv", [128, 2, H, DK + 1], FP8,
                             kind="ExternalOutput").ap()
        dod = nc.dram_tensor("do", [NHT, 128, 2, SQ], FP8,
                             kind="ExternalOutput").ap()
        dxd = nc.dram_tensor("dx", [128, ND, 256], BF16,
                             kind="ExternalOutput").ap()
        dzd = nc.dram_tensor("dz", [128, ND, 256], BF16,
                             kind="ExternalOutput").ap()
        dhd = nc.dram_tensor("dh", [128, NF, 256], BF16,
                             kind="ExternalOutput").ap()

    with tile.TileContext(nc) as tc, ExitStack() as ctx:
        const = ctx.enter_context(tc.tile_pool(name="const", bufs=1))

        eps_sb = const.tile([1, 1], F32, tag="eps")
        nc.vector.memset(eps_sb, EPS)
        nbias = const.tile([128, 1], F32, tag="nbias")
        nc.vector.memset(nbias, -2.0)
        ones_bf = const.tile([128, 1], BF16, tag="ones")
        nc.vector.memset(ones_bf, 1.0)

        ctxMLP = ExitStack()
        op8 = ctxMLP.enter_context(tc.tile_pool(name="op8", bufs=1))
        o8 = [op8.tile([128, 2, SQ], FP8, tag=f"o{t}", name=f"o8_{t}")
              for t in range(NHT)]

        # q/k feature-major bf16; v bf16 token-major (+ones col)
        ctxQK = ExitStack()
        qkp = ctxQK.enter_context(tc.tile_pool(name="qkp", bufs=1))
        vp = ctxQK.enter_context(tc.tile_pool(name="vp", bufs=1))
        q_dr = [qkp.tile([128, 2, SQ], FP8, tag=f"qz{h}", name=f"qz{h}")
                for h in range(H)]
        k_fm = qkp.tile([128, ND, S], FP8, tag="k", name="k_fm")
        v_all = vp.tile([128, NT, H, DK + 1], FP8, tag="v", name="v_all")

        ctxZW = ExitStack()
        zp = ctxZW.enter_context(tc.tile_pool(name="zp", bufs=1))
        wA = ctxZW.enter_context(tc.tile_pool(name="wA", bufs=1))
        # staged input loads: own z first (Q), then the rest
        z8 = [zp.tile([128, ND, 512], FP8, tag=f"z8q{c}", name=f"z8q{c}")
              for c in range(4)]
        nc.sync.dma_start(out=z8[0][:, 0:4, :], in_=z8d[0][:, 0:4, :])
        nc.scalar.dma_start(out=z8[0][:, 4:8, :], in_=z8d[0][:, 4:8, :])
        nc.sync.dma_start(out=z8[1], in_=z8d[1])
        nc.scalar.dma_start(out=z8[2], in_=z8d[2])
        nc.sync.dma_start(out=z8[3], in_=z8d[3])
        bq_sb = const.tile([128, ND], F32, tag="bq")
        nc.scalar.dma_start(out=bq_sb, in_=bqd)
        bo_sb = const.tile([128, ND], F32, tag="bo")
        nc.scalar.dma_start(out=bo_sb, in_=bod)
        b1_sb = const.tile([128, NF], F32, tag="b1")
        nc.scalar.dma_start(out=b1_sb, in_=b1d)
        b2_sb = const.tile([128, ND], F32, tag="b2")
        nc.scalar.dma_start(out=b2_sb, in_=b2d)

        wq8 = wA.tile([128, 4, 2, D], FP8, tag="wq8", name="wq8")
        nc.gpsimd.dma_start(out=wq8, in_=wq8d)
        wk8 = wA.tile([128, 4, 2, D], FP8, tag="wk8", name="wk8")
        nc.gpsimd.dma_start(out=wk8, in_=wk8d)
        wv8 = wA.tile([128, 4, 2, D], FP8, tag="wv8", name="wv8")
        nc.gpsimd.dma_start(out=wv8, in_=wv8d)
        wo8 = const.tile([128, 4, 2, D], FP8, tag="wo8", name="wo8")
        nc.gpsimd.dma_start(out=wo8, in_=wo8d)
        xfm = const.tile([128, ND, SQ], BF16, tag="xfm", name="xfm")
        nc.gpsimd.dma_start(out=xfm, in_=xfmd)

        ctxQKV = ExitStack()
        psA = ctxQKV.enter_context(tc.tile_pool(name="psA", bufs=4,
                                                space="PSUM"))

        # Q: own 512 tokens, fp8 DoubleRow; drains write per-head
        # zero-padded tiles so scores contract K=128 (no PE tile-config
        # switches vs the K=128 PV matmuls)
        for h in range(H):
            nc.gpsimd.memset(q_dr[h], 0.0)
        for j in range(ND):
            pq = psA.tile([128, 2, 512], F32, tag="ps")
            for c in range(4):
                nc.tensor.matmul(pq[:, 0, :],
                                 wq8[:, c, :, j * 128:(j + 1) * 128],
                                 z8[0][:, 2 * c:2 * c + 2, :],
                                 start=(c == 0), stop=(c == 3), perf_mode=DR)
            for s_ in range(2):
                h = 2 * j + s_
                nc.vector.tensor_scalar(
                    q_dr[h][64 * s_:64 * s_ + 64, j % 2, :],
                    pq[64 * s_:64 * s_ + 64, 0, :],
                    bq_sb[64 * s_:64 * s_ + 64, j:j + 1], None, op0=OP.add)

        # K: all 2048 tokens by quadrant; paired drains (no bias: bk
        # cancels per-query in softmax)
        for cq in range(4):
            for a in range(ND // 2):
                pk = psA.tile([128, 2, 512], F32, tag="ps")
                for half in range(2):
                    j = 2 * a + half
                    for c in range(4):
                        nc.tensor.matmul(
                            pk[:, half, :],
                            wk8[:, c, :, j * 128:(j + 1) * 128],
                            z8[cq][:, 2 * c:2 * c + 2, :],
                            start=(c == 0), stop=(c == 3), perf_mode=DR)
                nc.vector.tensor_copy(
                    k_fm[:, 2 * a:2 * a + 2, cq * 512:(cq + 1) * 512], pk)

        # V: token-major [tok, h, dk] bf16; paired drains
        nc.gpsimd.memset(v_all[:, :, :, DK:DK + 1], 1.0)
        for c in range(NT // 2):
            for half in range(2):
                pv = psA.tile([128, 2, 512], F32, tag="ps")
                for b_ in range(2):
                    t = 2 * c + b_
                    for d in range(4):
                        nc.tensor.matmul(
                            pv[:, b_, :],
                            z8[t // 4][:, 2 * d:2 * d + 2,
                                       (t % 4) * 128:(t % 4 + 1) * 128],
                            wv8[:, d, :, half * 512:(half + 1) * 512],
                            start=(d == 0), stop=(d == 3), perf_mode=DR)
                nc.vector.tensor_copy(
                    v_all[:, 2 * c:2 * c + 2, 8 * half:8 * half + 8, 0:DK],
                    pv.rearrange("p b (h d) -> p b h d", h=8))
        ctxQKV.close()
        ctxZW.close()

        # ---- attention: 16 heads, fp8 exp + DoubleRow PV ----
        ctxAPS = ExitStack()
        pgp = ctxAPS.enter_context(tc.tile_pool(name="pgp", bufs=3,
                                                space="PSUM"))
        ppvp = ctxAPS.enter_context(tc.tile_pool(name="ppvp", bufs=2,
                                                 space="PSUM"))
        stp = ctxAPS.enter_context(tc.tile_pool(name="stp", bufs=4))

        w1b = const.tile([128, ND, FF], BF16, tag="w1b", name="w1b")
        nc.gpsimd.dma_start(out=w1b, in_=w1bd)
        w2b = const.tile([128, NF, D], BF16, tag="w2b", name="w2b")
        nc.gpsimd.dma_start(out=w2b, in_=w2bd)

        for h in range(H):
            a2 = 2 * (h // 4)
            ppv = ppvp.tile([DK + 1, SQ], F32, tag="ppv", name=f"ppv{h}")
            for c in range(NT // 2):
                pg = pgp.tile([128, 2, 512], F32, tag="pg")
                for b_ in range(2):
                    kt = 2 * c + b_
                    nc.tensor.matmul(
                        pg[:, b_, :],
                        k_fm[:, a2:a2 + 2, kt * 128:(kt + 1) * 128],
                        q_dr[h], start=True, stop=True, perf_mode=DR)
                st8 = stp.tile([128, 2, 512], FP8, tag="st")
                nc.scalar.activation(st8, pg, AF.Exp, bias=nbias, scale=0.125)
                nc.tensor.matmul(ppv, v_all[:, 2 * c:2 * c + 2, h, :], st8,
                                 start=(c == 0), stop=(c == NT // 2 - 1),
                                 perf_mode=DR)
            den_c = stp.tile([1, SQ], F32, tag="denc", bufs=2)
            nc.vector.tensor_copy(den_c, ppv[DK:DK + 1, :])
            den_r = stp.tile([1, SQ], F32, tag="denr", bufs=2)
            nc.vector.reciprocal_approx_fast(den_r, den_c)
            rb = stp.tile([DK, SQ], F32, tag="rb", bufs=2)
            nc.gpsimd.partition_broadcast(rb, den_r)
            nc.vector.tensor_mul(
                o8[h // 4][64 * (h % 2):64 * (h % 2) + 64, (h // 2) % 2, :],
                ppv[0:DK, :], rb)

        ctxAPS.close()
        ctxQK.close()

        # ---- two token-halves: wo -> LN2 -> MLP pipeline ----
        psB = ctxMLP.enter_context(tc.tile_pool(name="psB", bufs=4,
                                                space="PSUM"))
        psST = ctxMLP.enter_context(tc.tile_pool(name="psST", bufs=2,
                                                 space="PSUM"))
        xrp = ctxMLP.enter_context(tc.tile_pool(name="xrp", bufs=1))
        lns = ctxMLP.enter_context(tc.tile_pool(name="lns", bufs=2))
        outp = ctxMLP.enter_context(tc.tile_pool(name="outp", bufs=1))
        x_res = [xrp.tile([128, ND, 256], BF16, tag=f"xr{s_}",
                          name=f"xres{s_}") for s_ in range(2)]
        xsq = [xrp.tile([128, ND, 256], BF16, tag=f"xq{s_}",
                        name=f"xsq{s_}") for s_ in range(2)]
        z2 = [xrp.tile([128, ND, 256], BF16, tag=f"z2{s_}",
                       name=f"z2_{s_}") for s_ in range(2)]
        h_sb = [xrp.tile([128, NF, 256], BF16, tag=f"h{s_}",
                         name=f"h_{s_}") for s_ in range(2)]
        out_tm = [outp.tile([128, 2, D], BF16, tag=f"otm{s_}",
                            name=f"out_tm{s_}") for s_ in range(2)]

        def wo_half(s_):
            lo = 256 * s_
            for o in range(ND):
                py = psB.tile([128, 256], F32, tag="psb")
                for c in range(NHT):
                    nc.tensor.matmul(py, wo8[:, c, :, o * 128:(o + 1) * 128],
                                     o8[c][:, :, lo:lo + 256],
                                     start=(c == 0), stop=(c == NHT - 1),
                                     perf_mode=DR)
                nc.vector.scalar_tensor_tensor(
                    x_res[s_][:, o, :], py, bo_sb[:, o:o + 1],
                    xfm[:, o, lo:lo + 256], op0=OP.add, op1=OP.add)
                nc.vector.tensor_mul(xsq[s_][:, o, :], x_res[s_][:, o, :],
                                     x_res[s_][:, o, :])

        stat_ps = {}

        def stats_half(s_):
            psum1 = psST.tile([1, 256], F32, tag="s1", bufs=2)
            pseq = psST.tile([1, 256], F32, tag="s2", bufs=2)
            stat_ps[s_] = (psum1, pseq)
            for o in range(ND):
                nc.tensor.matmul(psum1, ones_bf, x_res[s_][:, o, :],
                                 start=(o == 0), stop=(o == ND - 1))
            for o in range(ND):
                nc.tensor.matmul(pseq, ones_bf, xsq[s_][:, o, :],
                                 start=(o == 0), stop=(o == ND - 1))

        def ln2_head_half(s_):
            psum1, pseq = stat_ps[s_]
            mu = lns.tile([1, 256], F32, tag="mu", bufs=1)
            nc.vector.tensor_scalar(mu, psum1, 1.0 / D, None, op0=OP.mult)
            musq = lns.tile([1, 256], F32, tag="musq", bufs=1)
            nc.vector.tensor_mul(musq, mu, mu)
            var = lns.tile([1, 256], F32, tag="var", bufs=1)
            nc.vector.scalar_tensor_tensor(var, pseq, 1.0 / D, musq,
                                           op0=OP.mult, op1=OP.subtract)
            sq = lns.tile([1, 256], F32, tag="sq", bufs=1)
            nc.scalar.activation(sq, var, AF.Sqrt, bias=eps_sb, scale=1.0)
            rstd = lns.tile([1, 256], F32, tag="rstd", bufs=1)
            nc.vector.reciprocal(rstd, sq)
            mu_b = lns.tile([1, 256], BF16, tag="mub")
            nc.vector.tensor_copy(mu_b, mu)
            rstd_b = lns.tile([1, 256], BF16, tag="rstdb")
            nc.vector.tensor_copy(rstd_b, rstd)
            mu_bc = lns.tile([128, 256], BF16, tag="mubc")
            nc.gpsimd.partition_broadcast(mu_bc, mu_b)
            rstd_bc = lns.tile([128, 256], BF16, tag="rstdbc")
            nc.gpsimd.partition_broadcast(rstd_bc, rstd_b)
            nc.vector.tensor_sub(
                z2[s_], x_res[s_],
                mu_bc.rearrange("p (o t) -> p o t", o=1).broadcast_to(
                    [128, ND, 256]))
            nc.vector.tensor_mul(
                z2[s_], z2[s_],
                rstd_bc.rearrange("p (o t) -> p o t", o=1).broadcast_to(
                    [128, ND, 256]))

        def mlp_body_half(s_):
            lo = 256 * s_
            # MLP1 + relu(+b1) -> bf16 h
            for f in range(NF):
                ph = psB.tile([128, 256], F32, tag="psb")
                for d in range(ND):
                    nc.tensor.matmul(ph, w1b[:, d, f * 128:(f + 1) * 128],
                                     z2[s_][:, d, :],
                                     start=(d == 0), stop=(d == ND - 1))
                nc.scalar.activation(h_sb[s_][:, f, :], ph, AF.Relu,
                                     bias=b1_sb[:, f:f + 1], scale=1.0)
            # MLP2 + b2 + x_res -> bf16 out_fm -> transpose to token-major
            for o in range(ND):
                p2 = psB.tile([128, 256], F32, tag="psb")
                for f in range(NF):
                    nc.tensor.matmul(p2, w2b[:, f, o * 128:(o + 1) * 128],
                                     h_sb[s_][:, f, :],
                                     start=(f == 0), stop=(f == NF - 1))
                ofm = lns.tile([128, 256], BF16, tag="ofm", bufs=2)
                nc.vector.scalar_tensor_tensor(
                    ofm, p2, b2_sb[:, o:o + 1], x_res[s_][:, o, :],
                    op0=OP.add, op1=OP.add)
                nc.sync.dma_start_transpose(
                    out_tm[s_][:, :, o * 128:(o + 1) * 128], ofm)
            for tt in range(2):
                t = 2 * s_ + tt
                o_st = outp.tile([128, D], F32, tag="ost", bufs=2)
                nc.vector.tensor_copy(o_st, out_tm[s_][:, tt, :])
                eng = nc.sync if t % 2 == 0 else nc.gpsimd
                eng.dma_start(out=outd[t * 128:(t + 1) * 128, :], in_=o_st)

        wo_half(0)
        stats_half(0)
        ln2_head_half(0)
        wo_half(1)
        stats_half(1)
        mlp_body_half(0)
        ln2_head_half(1)
        mlp_body_half(1)

        _dbg_marker = True
        if dbg:
            nc.sync.dma_start(out=dqd[:, 0:2, :].rearrange("p a t -> p (a t)"), in_=q_z[0])
            nc.sync.dma_start(out=dkd, in_=k_fm)
            nc.sync.dma_start(out=dvd, in_=v_all[:, 0:2, :, :])
            for t in range(NHT):
                nc.sync.dma_start(out=dod[t], in_=o8[t])
            nc.sync.dma_start(out=dxd, in_=x_res[0])
            nc.sync.dma_start(out=dzd, in_=z2[0])
            nc.sync.dma_start(out=dhd, in_=h_sb[0])
        ctxMLP.close()

    nc.compile()
    return nc


_LOCK = threading.Lock()
_NC = None


def _get_nc():
    global _NC
    with _LOCK:
        if _NC is None:
            _NC = _build_nc()
    return _NC


def _prep_inputs(inputs):
    x = np.asarray(inputs["x"], np.float32)
    g1 = np.asarray(inputs["ln1_g"], np.float32)
    lb1 = np.asarray(inputs["ln1_b"], np.float32)
    g2 = np.asarray(inputs["ln2_g"], np.float32)
    lb2 = np.asarray(inputs["ln2_b"], np.float32)
    wq = np.asarray(inputs["wq"], np.float32)
    wk = np.asarray(inputs["wk"], np.float32)
    wv = np.asarray(inputs["wv"], np.float32)
    wo = np.asarray(inputs["wo"], np.float32)
    w1 = np.asarray(inputs["w1"], np.float32)
    w2 = np.asarray(inputs["w2"], np.float32)

    def dr8(wt):
        # [D_in, D_out] -> [128, 4, 2, D_out] fp8 DoubleRow layout
        return np.ascontiguousarray(
            wt.reshape(4, 2, 128, D).transpose(2, 0, 1, 3)).astype(_F8)

    # host LN1 + fp8 quantize, feature-major
    mu = x.mean(-1, keepdims=True)
    var = x.var(-1, keepdims=True)
    z = (x - mu) / np.sqrt(var + EPS)          # [B, S, D]
    z8 = z.transpose(0, 2, 1).astype(_F8)      # [B, D, S] feature-major
    xfm_all = x.transpose(0, 2, 1).astype(_BF)  # [B, D, S]

    shared = {
        "wq8": dr8(g1[:, None] * wq.T),
        "wk8": dr8(g1[:, None] * wk.T),
        "wv8": dr8(g1[:, None] * wv.T),
        "wo8": dr8(wo.T),
        "w1b": np.ascontiguousarray(
            (g2[:, None] * w1.T).reshape(ND, 128, FF).transpose(
                1, 0, 2)).astype(_BF),
        "w2b": np.ascontiguousarray(
            w2.T.reshape(NF, 128, D).transpose(1, 0, 2)).astype(_BF),
        "bq": np.ascontiguousarray(
            (np.asarray(inputs["bq"], np.float32) + wq @ lb1).reshape(
                ND, 128).T),
        "bo": np.ascontiguousarray(
            (np.asarray(inputs["bo"], np.float32)
             + wo @ np.asarray(inputs["bv"], np.float32)).reshape(
                 ND, 128).T),
        "b1": np.ascontiguousarray(
            (np.asarray(inputs["b1"], np.float32) + w1 @ lb2).reshape(
                NF, 128).T),
        "b2": np.ascontiguousarray(
            np.asarray(inputs["b2"], np.float32).reshape(ND, 128).T),
    }

    in_maps = []
    for core in range(NCORES):
        b = core // (NCORES // B)
        qoff = (core % (NCORES // B)) * SQ
        zb = z8[b]                              # [D, S] fp8
        # own 512 tokens first, then the rest (key order is softmax-inv)
        perm = np.concatenate(
            [np.arange(qoff, qoff + SQ), np.arange(0, qoff),
             np.arange(qoff + SQ, S)])
        zperm = zb[:, perm]                     # [D, S]
        m = dict(shared)
        for c in range(4):
            m[f"z8q{c}"] = np.ascontiguousarray(
                zperm[:, c * 512:(c + 1) * 512].reshape(ND, 128, 512)
                .transpose(1, 0, 2))
        m["xfm"] = np.ascontiguousarray(
            xfm_all[b][:, qoff:qoff + SQ].reshape(ND, 128, SQ)
            .transpose(1, 0, 2))
        in_maps.append(m)
    return in_maps


def _run(inputs, trace=False, tmpdir=None):
    nc = _get_nc()
    in_maps = _prep_inputs(inputs)
    res = run_bass_kernel_spmd(nc, in_maps, core_ids=list(range(NCORES)),
                               trace=trace, tmpdir=tmpdir)
    out = np.empty((B, S, D), np.float32)
    for core in range(NCORES):
        b = core // (NCORES // B)
        qoff = (core % (NCORES // B)) * SQ
        out[b, qoff:qoff + SQ] = res.results[core]["out"]
    return out, res


def kernel(**inputs):
    out, _ = _run(inputs, trace=False)
    return out


# revision 23
# speedup vs baseline: 1.0268x; 1.0263x over previous
"""Trainium2 Bass kernel for a pre-norm transformer decoder layer.

Full inputs in, full output out. 8-way data-parallel over tokens
(batch 2 x 4 query-slices of 512). Each core redundantly computes K/V
for its batch's full 2048-token sequence and owns 512 query tokens.

v2 design:
- Host precomputes LN1 (mu/rstd) and uploads z8 = fp8(norm(x)) in a
  feature-major layout, plus fp8 DoubleRow-interleaved attention
  weights (2x tensor throughput on contraction >= 256).
- Attention entirely in fp8: Q/K/V/out-proj via fp8 DoubleRow matmuls;
  exp(scores) written as fp8 with a -2 bias (cancels in softmax);
  PV contracts 256 keys per DoubleRow matmul with an appended
  ones-column accumulating the softmax denominator.
- bk cancels in softmax (per-query shift); bv folds into bo; bq rides
  the PSUM-drain bias; ln biases fold into bq/b1 (host).
- MLP stays bf16 (fp8 there costs ~1.7e-2 rel err; attention fp8 costs
  ~1e-3). LN2 stats via ones-matmul over feature-major x_res.
- Two-stage token pipeline (256+256) overlaps wo/LN2/MLP tensor work
  with the scalar-engine-bound exp stream of the next attention half.

Shapes: x (2, 2048, 1024), 16 heads, dk=64, d_ff=2048, eps=1e-5.
"""
import os
import threading

import numpy as np
import ml_dtypes

import concourse.mybir as mybir
import concourse.tile as tile
from concourse import bacc
from concourse.bass_utils import run_bass_kernel_spmd
from contextlib import ExitStack

F32 = mybir.dt.float32
BF16 = mybir.dt.bfloat16
FP8 = mybir.dt.float8e4
AF = mybir.ActivationFunctionType
OP = mybir.AluOpType
DR = mybir.MatmulPerfMode.DoubleRow

B, S, D = 2, 2048, 1024
H, DK, FF = 16, 64, 2048
EPS = 1e-5
NCORES = 8
SQ = S * B // NCORES          # 512 own query tokens per core
ND = D // 128                 # 8 feature chunks
NT = S // 128                 # 16 key-token tiles
NF = FF // 128                # 16 ff chunks
NHT = 4                       # o8 tiles (4 heads each)

_BF = ml_dtypes.bfloat16
_F8 = ml_dtypes.float8_e4m3fn


def _build_nc():
    nc = bacc.Bacc("TRN2", target_bir_lowering=False, debug=False,
                   num_devices=NCORES)

    z8d = [nc.dram_tensor(f"z8q{c}", [128, ND, 512], FP8,
                          kind="ExternalInput").ap() for c in range(4)]
    xfmd = nc.dram_tensor("xfm", [128, ND, SQ], BF16,
                          kind="ExternalInput").ap()
    wq8d = nc.dram_tensor("wq8", [128, 4, 2, D], FP8,
                          kind="ExternalInput").ap()
    wk8d = nc.dram_tensor("wk8", [128, 4, 2, D], FP8,
                          kind="ExternalInput").ap()
    wv8d = nc.dram_tensor("wv8", [128, 4, 2, D], FP8,
                          kind="ExternalInput").ap()
    wo8d = nc.dram_tensor("wo8", [128, 4, 2, D], FP8,
                          kind="ExternalInput").ap()
    w1bd = nc.dram_tensor("w1b", [128, ND, FF], BF16,
                          kind="ExternalInput").ap()
    w2bd = nc.dram_tensor("w2b", [128, NF, D], BF16,
                          kind="ExternalInput").ap()
    bqd = nc.dram_tensor("bq", [128, ND], F32, kind="ExternalInput").ap()
    bod = nc.dram_tensor("bo", [128, ND], F32, kind="ExternalInput").ap()
    b1d = nc.dram_tensor("b1", [128, NF], F32, kind="ExternalInput").ap()
    b2d = nc.dram_tensor("b2", [128, ND], F32, kind="ExternalInput").ap()
    outd = nc.dram_tensor("out", [SQ, D], F32, kind="ExternalOutput").ap()
    dbg = os.environ.get("KDBG", "0") == "1"
    if dbg:
        dqd = nc.dram_tensor("dq", [128, ND, SQ], BF16,
                             kind="ExternalOutput").ap()
        dkd = nc.dram_tensor("dk", [128, ND, S], BF16,
                             kind="ExternalOutput").ap()
        dvd = nc.dram_tensor("dv", [128, 2, H, DK + 1], FP8,
                             kind="ExternalOutput").ap()
        dod = nc.dram_tensor("do", [NHT, 128, 2, SQ], FP8,
                             kind="ExternalOutput").ap()
        dxd = nc.dram_tensor("dx", [128, ND, 256], BF16,
                             kind="ExternalOutput").ap()
        dzd = nc.dram_tensor("dz", [128, ND, 256], BF16,
                             kind="ExternalOutput").ap()
        dhd = nc.dram_tensor("dh", [128, NF, 256], BF16,
                             kind="ExternalOutput").ap()

    with tile.TileContext(nc) as tc, ExitStack() as ctx:
        const = ctx.enter_context(tc.tile_pool(name="const", bufs=1))

        eps_sb = const.tile([1, 1], F32, tag="eps")
        nc.vector.memset(eps_sb, EPS)
        nbias = const.tile([128, 1], F32, tag="nbias")
        nc.vector.memset(nbias, -2.0)
        ones_bf = const.tile([128, 128], BF16, tag="ones")
        nc.vector.memset(ones_bf, 1.0)

        ctxMLP = ExitStack()
        op8 = ctxMLP.enter_context(tc.tile_pool(name="op8", bufs=1))
        o8 = [op8.tile([128, 2, SQ], FP8, tag=f"o{t}", name=f"o8_{t}")
              for t in range(NHT)]

        # q/k feature-major bf16; v bf16 token-major (+ones col)
        ctxQK = ExitStack()
        qkp = ctxQK.enter_context(tc.tile_pool(name="qkp", bufs=1))
        vp = ctxQK.enter_context(tc.tile_pool(name="vp", bufs=1))
        q_dr = [qkp.tile([128, 2, SQ], FP8, tag=f"qz{h}", name=f"qz{h}")
                for h in range(H)]
        k_fm = qkp.tile([128, ND, S], FP8, tag="k", name="k_fm")
        v_all = vp.tile([128, NT, H, DK + 1], FP8, tag="v", name="v_all")

        ctxZW = ExitStack()
        zp = ctxZW.enter_context(tc.tile_pool(name="zp", bufs=1))
        wA = ctxZW.enter_context(tc.tile_pool(name="wA", bufs=1))
        # staged input loads: own z first (Q), then the rest
        z8 = [zp.tile([128, ND, 512], FP8, tag=f"z8q{c}", name=f"z8q{c}")
              for c in range(4)]
        nc.sync.dma_start(out=z8[0][:, 0:4, :], in_=z8d[0][:, 0:4, :])
        nc.scalar.dma_start(out=z8[0][:, 4:8, :], in_=z8d[0][:, 4:8, :])
        nc.sync.dma_start(out=z8[1], in_=z8d[1])
        nc.scalar.dma_start(out=z8[2], in_=z8d[2])
        nc.sync.dma_start(out=z8[3], in_=z8d[3])
        bq_sb = const.tile([128, ND], F32, tag="bq")
        nc.scalar.dma_start(out=bq_sb, in_=bqd)
        bo_sb = const.tile([128, ND], F32, tag="bo")
        nc.scalar.dma_start(out=bo_sb, in_=bod)
        b1_sb = const.tile([128, NF], F32, tag="b1")
        nc.scalar.dma_start(out=b1_sb, in_=b1d)
        b2_sb = const.tile([128, ND], F32, tag="b2")
        nc.scalar.dma_start(out=b2_sb, in_=b2d)

        wq8 = wA.tile([128, 4, 2, D], FP8, tag="wq8", name="wq8")
        nc.gpsimd.dma_start(out=wq8[:, :, :, 0:256], in_=wq8d[:, :, :, 0:256])
        nc.gpsimd.dma_start(out=wq8[:, :, :, 256:D],
                            in_=wq8d[:, :, :, 256:D])
        wk8 = wA.tile([128, 4, 2, D], FP8, tag="wk8", name="wk8")
        nc.gpsimd.dma_start(out=wk8, in_=wk8d)
        wv8 = wA.tile([128, 4, 2, D], FP8, tag="wv8", name="wv8")
        nc.gpsimd.dma_start(out=wv8, in_=wv8d)
        wo8 = const.tile([128, 4, 2, D], FP8, tag="wo8", name="wo8")
        nc.gpsimd.dma_start(out=wo8, in_=wo8d)
        xfm = const.tile([128, ND, SQ], BF16, tag="xfm", name="xfm")
        nc.gpsimd.dma_start(out=xfm, in_=xfmd)

        ctxQKV = ExitStack()
        psA = ctxQKV.enter_context(tc.tile_pool(name="psA", bufs=4,
                                                space="PSUM"))

        # Q: own 512 tokens, fp8 DoubleRow; drains write per-head
        # zero-padded tiles so scores contract K=128 (no PE tile-config
        # switches vs the K=128 PV matmuls)
        for h in range(H):
            nc.gpsimd.memset(q_dr[h], 0.0)
        for j in range(ND):
            pq = psA.tile([128, 2, 512], F32, tag="ps")
            for c in range(4):
                nc.tensor.matmul(pq[:, 0, :],
                                 wq8[:, c, :, j * 128:(j + 1) * 128],
                                 z8[0][:, 2 * c:2 * c + 2, :],
                                 start=(c == 0), stop=(c == 3), perf_mode=DR)
            for s_ in range(2):
                h = 2 * j + s_
                nc.vector.tensor_scalar(
                    q_dr[h][64 * s_:64 * s_ + 64, j % 2, :],
                    pq[64 * s_:64 * s_ + 64, 0, :],
                    bq_sb[64 * s_:64 * s_ + 64, j:j + 1], None, op0=OP.add)

        # K: all 2048 tokens by quadrant; paired drains (no bias: bk
        # cancels per-query in softmax)
        for cq in range(4):
            for a in range(ND // 2):
                pk = psA.tile([128, 2, 512], F32, tag="ps")
                for half in range(2):
                    j = 2 * a + half
                    for c in range(4):
                        nc.tensor.matmul(
                            pk[:, half, :],
                            wk8[:, c, :, j * 128:(j + 1) * 128],
                            z8[cq][:, 2 * c:2 * c + 2, :],
                            start=(c == 0), stop=(c == 3), perf_mode=DR)
                nc.vector.tensor_copy(
                    k_fm[:, 2 * a:2 * a + 2, cq * 512:(cq + 1) * 512], pk)

        # V: token-major [tok, h, dk] bf16; paired drains
        nc.gpsimd.memset(v_all[:, :, :, DK:DK + 1], 1.0)
        for c in range(NT // 2):
            for half in range(2):
                pv = psA.tile([128, 2, 512], F32, tag="ps")
                for b_ in range(2):
                    t = 2 * c + b_
                    for d in range(4):
                        nc.tensor.matmul(
                            pv[:, b_, :],
                            z8[t // 4][:, 2 * d:2 * d + 2,
                                       (t % 4) * 128:(t % 4 + 1) * 128],
                            wv8[:, d, :, half * 512:(half + 1) * 512],
                            start=(d == 0), stop=(d == 3), perf_mode=DR)
                nc.vector.tensor_copy(
                    v_all[:, 2 * c:2 * c + 2, 8 * half:8 * half + 8, 0:DK],
                    pv.rearrange("p b (h d) -> p b h d", h=8))
        ctxQKV.close()
        ctxZW.close()

        # ---- attention: 16 heads, fp8 exp + DoubleRow PV ----
        ctxAPS = ExitStack()
        pgp = ctxAPS.enter_context(tc.tile_pool(name="pgp", bufs=3,
                                                space="PSUM"))
        ppvp = ctxAPS.enter_context(tc.tile_pool(name="ppvp", bufs=2,
                                                 space="PSUM"))
        stp = ctxAPS.enter_context(tc.tile_pool(name="stp", bufs=4))

        w1b = const.tile([128, ND, FF], BF16, tag="w1b", name="w1b")
        nc.gpsimd.dma_start(out=w1b, in_=w1bd)
        w2b = const.tile([128, NF, D], BF16, tag="w2b", name="w2b")
        nc.gpsimd.dma_start(out=w2b, in_=w2bd)

        for h in range(H):
            a2 = 2 * (h // 4)
            ppv = ppvp.tile([DK + 1, SQ], F32, tag="ppv", name=f"ppv{h}")
            for c in range(NT // 2):
                pg = pgp.tile([128, 2, 512], F32, tag="pg")
                for b_ in range(2):
                    kt = 2 * c + b_
                    nc.tensor.matmul(
                        pg[:, b_, :],
                        k_fm[:, a2:a2 + 2, kt * 128:(kt + 1) * 128],
                        q_dr[h], start=True, stop=True, perf_mode=DR)
                st8 = stp.tile([128, 2, 512], FP8, tag="st")
                nc.scalar.activation(st8, pg, AF.Exp, bias=nbias, scale=0.125)
                nc.tensor.matmul(ppv, v_all[:, 2 * c:2 * c + 2, h, :], st8,
                                 start=(c == 0), stop=(c == NT // 2 - 1),
                                 perf_mode=DR)
            den_c = stp.tile([1, SQ], F32, tag="denc", bufs=2)
            nc.vector.tensor_copy(den_c, ppv[DK:DK + 1, :])
            den_r = stp.tile([1, SQ], F32, tag="denr", bufs=2)
            nc.vector.reciprocal_approx_fast(den_r, den_c)
            rb = stp.tile([DK, SQ], F32, tag="rb", bufs=2)
            nc.gpsimd.partition_broadcast(rb, den_r)
            nc.vector.tensor_mul(
                o8[h // 4][64 * (h % 2):64 * (h % 2) + 64, (h // 2) % 2, :],
                ppv[0:DK, :], rb)

        ctxAPS.close()
        ctxQK.close()

        # ---- two token-halves: wo -> LN2 -> MLP pipeline ----
        psB = ctxMLP.enter_context(tc.tile_pool(name="psB", bufs=4,
                                                space="PSUM"))
        psST = ctxMLP.enter_context(tc.tile_pool(name="psST", bufs=2,
                                                 space="PSUM"))
        xrp = ctxMLP.enter_context(tc.tile_pool(name="xrp", bufs=1))
        lns = ctxMLP.enter_context(tc.tile_pool(name="lns", bufs=2))
        outp = ctxMLP.enter_context(tc.tile_pool(name="outp", bufs=1))
        x_res = [xrp.tile([128, ND, 256], BF16, tag=f"xr{s_}",
                          name=f"xres{s_}") for s_ in range(2)]
        xsq = [xrp.tile([128, ND, 256], BF16, tag=f"xq{s_}",
                        name=f"xsq{s_}") for s_ in range(2)]
        z2 = [xrp.tile([128, ND, 256], BF16, tag=f"z2{s_}",
                       name=f"z2_{s_}") for s_ in range(2)]
        h_sb = [xrp.tile([128, NF, 256], BF16, tag=f"h{s_}",
                         name=f"h_{s_}") for s_ in range(2)]
        out_tm = [outp.tile([128, 2, D], BF16, tag=f"otm{s_}",
                            name=f"out_tm{s_}") for s_ in range(2)]

        def wo_half(s_):
            lo = 256 * s_
            for o in range(ND):
                py = psB.tile([128, 256], F32, tag="psb")
                for c in range(NHT):
                    nc.tensor.matmul(py, wo8[:, c, :, o * 128:(o + 1) * 128],
                                     o8[c][:, :, lo:lo + 256],
                                     start=(c == 0), stop=(c == NHT - 1),
                                     perf_mode=DR)
                nc.vector.scalar_tensor_tensor(
                    x_res[s_][:, o, :], py, bo_sb[:, o:o + 1],
                    xfm[:, o, lo:lo + 256], op0=OP.add, op1=OP.add)
                nc.vector.tensor_mul(xsq[s_][:, o, :], x_res[s_][:, o, :],
                                     x_res[s_][:, o, :])

        stat_ps = {}

        def stats_half(s_):
            psum1 = psST.tile([128, 256], F32, tag="s1", bufs=2)
            pseq = psST.tile([128, 256], F32, tag="s2", bufs=2)
            stat_ps[s_] = (psum1, pseq)
            for o in range(ND):
                nc.tensor.matmul(psum1, ones_bf, x_res[s_][:, o, :],
                                 start=(o == 0), stop=(o == ND - 1))
            for o in range(ND):
                nc.tensor.matmul(pseq, ones_bf, xsq[s_][:, o, :],
                                 start=(o == 0), stop=(o == ND - 1))

        def ln2_head_half(s_):
            psum1, pseq = stat_ps[s_]
            mu = lns.tile([1, 256], F32, tag="mu", bufs=1)
            nc.vector.tensor_scalar(mu, psum1[0:1, :], 1.0 / D, None,
                                    op0=OP.mult)
            musq = lns.tile([1, 256], F32, tag="musq", bufs=1)
            nc.vector.tensor_mul(musq, mu, mu)
            var = lns.tile([1, 256], F32, tag="var", bufs=1)
            nc.vector.scalar_tensor_tensor(var, pseq[0:1, :], 1.0 / D,
                                           musq, op0=OP.mult,
                                           op1=OP.subtract)
            sq = lns.tile([1, 256], F32, tag="sq", bufs=1)
            nc.scalar.activation(sq, var, AF.Sqrt, bias=eps_sb, scale=1.0)
            rstd = lns.tile([1, 256], F32, tag="rstd", bufs=1)
            nc.vector.reciprocal(rstd, sq)
            mu_b = lns.tile([1, 256], BF16, tag="mub")
            nc.vector.tensor_copy(mu_b, mu)
            rstd_b = lns.tile([1, 256], BF16, tag="rstdb")
            nc.vector.tensor_copy(rstd_b, rstd)
            mu_bc = lns.tile([128, 256], BF16, tag="mubc")
            nc.gpsimd.partition_broadcast(mu_bc, mu_b)
            rstd_bc = lns.tile([128, 256], BF16, tag="rstdbc")
            nc.gpsimd.partition_broadcast(rstd_bc, rstd_b)
            nc.vector.tensor_sub(
                z2[s_], x_res[s_],
                mu_bc.rearrange("p (o t) -> p o t", o=1).broadcast_to(
                    [128, ND, 256]))
            nc.vector.tensor_mul(
                z2[s_], z2[s_],
                rstd_bc.rearrange("p (o t) -> p o t", o=1).broadcast_to(
                    [128, ND, 256]))

        def mlp_body_half(s_):
            lo = 256 * s_
            # MLP1 + relu(+b1) -> bf16 h
            for f in range(NF):
                ph = psB.tile([128, 256], F32, tag="psb")
                for d in range(ND):
                    nc.tensor.matmul(ph, w1b[:, d, f * 128:(f + 1) * 128],
                                     z2[s_][:, d, :],
                                     start=(d == 0), stop=(d == ND - 1))
                nc.scalar.activation(h_sb[s_][:, f, :], ph, AF.Relu,
                                     bias=b1_sb[:, f:f + 1], scale=1.0)
            # MLP2 + b2 + x_res -> bf16 out_fm -> transpose to token-major
            for o in range(ND):
                p2 = psB.tile([128, 256], F32, tag="psb")
                for f in range(NF):
                    nc.tensor.matmul(p2, w2b[:, f, o * 128:(o + 1) * 128],
                                     h_sb[s_][:, f, :],
                                     start=(f == 0), stop=(f == NF - 1))
                ofm = lns.tile([128, 256], BF16, tag="ofm", bufs=2)
                nc.vector.scalar_tensor_tensor(
                    ofm, p2, b2_sb[:, o:o + 1], x_res[s_][:, o, :],
                    op0=OP.add, op1=OP.add)
                nc.sync.dma_start_transpose(
                    out_tm[s_][:, :, o * 128:(o + 1) * 128], ofm)
            for tt in range(2):
                t = 2 * s_ + tt
                o_st = outp.tile([128, D], F32, tag="ost", bufs=2)
                nc.vector.tensor_copy(o_st, out_tm[s_][:, tt, :])
                eng = nc.sync if t % 2 == 0 else nc.gpsimd
                eng.dma_start(out=outd[t * 128:(t + 1) * 128, :], in_=o_st)

        wo_half(0)
        stats_half(0)
        ln2_head_half(0)
        wo_half(1)
        stats_half(1)
        mlp_body_half(0)
        ln2_head_half(1)
        mlp_body_half(1)

        _dbg_marker = True
        if dbg:
            nc.sync.dma_start(out=dqd[:, 0:2, :].rearrange("p a t -> p (a t)"), in_=q_z[0])
            nc.sync.dma_start(out=dkd, in_=k_fm)
            nc.sync.dma_start(out=dvd, in_=v_all[:, 0:2, :, :])
            for t in range(NHT):
                nc.sync.dma_start(out=dod[t], in_=o8[t])
            nc.sync.dma_start(out=dxd, in_=x_res[0])
            nc.sync.dma_start(out=dzd, in_=z2[0])
            nc.sync.dma_start(out=dhd, in_=h_sb[0])
        ctxMLP.close()

    nc.compile()
    return nc


_LOCK = threading.Lock()
_NC = None


def _get_nc():
    global _NC
    with _LOCK:
        if _NC is None:
            _NC = _build_nc()
    return _NC


def _prep_inputs(inputs):
    x = np.asarray(inputs["x"], np.float32)
    g1 = np.asarray(inputs["ln1_g"], np.float32)
    lb1 = np.asarray(inputs["ln1_b"], np.float32)
    g2 = np.asarray(inputs["ln2_g"], np.float32)
    lb2 = np.asarray(inputs["ln2_b"], np.float32)
    wq = np.asarray(inputs["wq"], np.float32)
    wk = np.asarray(inputs["wk"], np.float32)
    wv = np.asarray(inputs["wv"], np.float32)
    wo = np.asarray(inputs["wo"], np.float32)
    w1 = np.asarray(inputs["w1"], np.float32)
    w2 = np.asarray(inputs["w2"], np.float32)

    def dr8(wt):
        # [D_in, D_out] -> [128, 4, 2, D_out] fp8 DoubleRow layout
        return np.ascontiguousarray(
            wt.reshape(4, 2, 128, D).transpose(2, 0, 1, 3)).astype(_F8)

    # host LN1 + fp8 quantize, feature-major
    mu = x.mean(-1, keepdims=True)
    var = x.var(-1, keepdims=True)
    z = (x - mu) / np.sqrt(var + EPS)          # [B, S, D]
    z8 = z.transpose(0, 2, 1).astype(_F8)      # [B, D, S] feature-major
    xfm_all = x.transpose(0, 2, 1).astype(_BF)  # [B, D, S]

    shared = {
        "wq8": dr8(g1[:, None] * wq.T),
        "wk8": dr8(g1[:, None] * wk.T),
        "wv8": dr8(g1[:, None] * wv.T),
        "wo8": dr8(wo.T),
        "w1b": np.ascontiguousarray(
            (g2[:, None] * w1.T).reshape(ND, 128, FF).transpose(
                1, 0, 2)).astype(_BF),
        "w2b": np.ascontiguousarray(
            w2.T.reshape(NF, 128, D).transpose(1, 0, 2)).astype(_BF),
        "bq": np.ascontiguousarray(
            (np.asarray(inputs["bq"], np.float32) + wq @ lb1).reshape(
                ND, 128).T),
        "bo": np.ascontiguousarray(
            (np.asarray(inputs["bo"], np.float32)
             + wo @ np.asarray(inputs["bv"], np.float32)).reshape(
                 ND, 128).T),
        "b1": np.ascontiguousarray(
            (np.asarray(inputs["b1"], np.float32) + w1 @ lb2).reshape(
                NF, 128).T),
        "b2": np.ascontiguousarray(
            np.asarray(inputs["b2"], np.float32).reshape(ND, 128).T),
    }

    in_maps = []
    for core in range(NCORES):
        b = core // (NCORES // B)
        qoff = (core % (NCORES // B)) * SQ
        zb = z8[b]                              # [D, S] fp8
        # own 512 tokens first, then the rest (key order is softmax-inv)
        perm = np.concatenate(
            [np.arange(qoff, qoff + SQ), np.arange(0, qoff),
             np.arange(qoff + SQ, S)])
        zperm = zb[:, perm]                     # [D, S]
        m = dict(shared)
        for c in range(4):
            m[f"z8q{c}"] = np.ascontiguousarray(
                zperm[:, c * 512:(c + 1) * 512].reshape(ND, 128, 512)
                .transpose(1, 0, 2))
        m["xfm"] = np.ascontiguousarray(
            xfm_all[b][:, qoff:qoff + SQ].reshape(ND, 128, SQ)
            .transpose(1, 0, 2))
        in_maps.append(m)
    return in_maps


def _run(inputs, trace=False, tmpdir=None):
    nc = _get_nc()
    in_maps = _prep_inputs(inputs)
    res = run_bass_kernel_spmd(nc, in_maps, core_ids=list(range(NCORES)),
                               trace=trace, tmpdir=tmpdir)
    out = np.empty((B, S, D), np.float32)
    for core in range(NCORES):
        b = core // (NCORES // B)
        qoff = (core % (NCORES // B)) * SQ
        out[b, qoff:qoff + SQ] = res.results[core]["out"]
    return out, res


def kernel(**inputs):
    out, _ = _run(inputs, trace=False)
    return out
